# revision 1
# baseline (speedup 1.0000x reference)
"""Trainium2 Bass kernel for EvaAttention (B=4, S=2048, C=1024, H=16, D=64).

Sharding: 8 cores = 4 batches x 2 head-groups (8 heads each). Each core runs
the identical SPMD program on host-sliced inputs:
  - qk-projection in [chan, seq] layout (2 heads per 128-partition tile),
    RoPE applied via a permutation matmul (rotate-half) + fused DVE ops,
  - v-projection in [seq, chan] layout with a packed ones-column so the
    attention matmul also produces the softmax denominators,
  - per-head  exp(QK^T) -> AV accumulate -> normalize,
  - output projection producing the per-core partial y^T.
Host sums the two head-group partials per batch and adds the bias
corrections (proj bias + v_bias folded through the projection).
"""

import os
import sys

import numpy as np

for _p in ("/opt/trn_rl_repo", "/root/.axon_site/_ro/trn_rl_repo"):
    if os.path.isdir(_p) and _p not in sys.path:
        sys.path.append(_p)

import concourse.bass as bass  # noqa: E402,F401
import concourse.mybir as mybir  # noqa: E402
import concourse.tile as tile  # noqa: E402
from concourse import bacc  # noqa: E402
from concourse.bass_utils import run_bass_kernel_spmd  # noqa: E402

F32 = mybir.dt.float32
F32R = mybir.dt.float32r
AF = mybir.ActivationFunctionType
OP = mybir.AluOpType

B = 4
C = 1024
D = 64
H = 16
HPC = 8  # heads per core
NCORES = 8
KC = C // 128  # contraction chunks for the projections
VW = D + 1  # v-store block width per head (64 v cols + ones col)


def _nchunks(width):
    """Split a free-dim width into <=512 column chunks."""
    out = []
    n0 = 0
    while n0 < width:
        nn = min(512, width - n0)
        out.append((n0, nn))
        n0 += nn
    return out


def _emit(tc, io, S):
    nc = tc.nc
    KT = S // 128  # k-position tiles
    S2 = S // 2  # attention q-pass width
    full_chunks = _nchunks(S)  # xt streaming quarters / output col chunks
    half_chunks = _nchunks(S2)

    from contextlib import ExitStack

    with (
        tc.tile_pool(name="pairp", bufs=1) as pair_pool,
        tc.tile_pool(name="pwp", bufs=1) as pw_pool,
        ExitStack() as transient,
    ):
        qkf_pool = transient.enter_context(tc.tile_pool(name="qkfp", bufs=1))
        v_pool = transient.enter_context(tc.tile_pool(name="vstp", bufs=1))
        xt_pool = transient.enter_context(tc.tile_pool(name="xtp", bufs=1))
        wqk_pool = transient.enter_context(tc.tile_pool(name="wqkp", bufs=1))
        cpool = transient.enter_context(tc.tile_pool(name="cstp", bufs=1))
        rope_pool = transient.enter_context(tc.tile_pool(name="ropep", bufs=1))
        attn_pool = transient.enter_context(tc.tile_pool(name="attnp", bufs=1))
        div_pool = transient.enter_context(tc.tile_pool(name="divp", bufs=1))
        oh_pool = transient.enter_context(tc.tile_pool(name="ohp", bufs=1))
        # ---- constants -------------------------------------------------
        qkb_sb = cpool.tile([128, 8], F32, tag="qkb", name="qkb")
        nc.sync.dma_start(out=qkb_sb, in_=io["qkb"])
        qkbr_sb = cpool.tile([128, 8], F32, tag="qkbr", name="qkbr")
        nc.sync.dma_start(out=qkbr_sb, in_=io["qkbr"])
        cos2_sb = cpool.tile([128, S], F32, tag="cos2", name="cos2")
        sin2_sb = cpool.tile([128, S], F32, tag="sin2", name="sin2")
        v_store = v_pool.tile([128, KT * HPC * VW], F32R, tag="vst", name="vst")

        psum_stack = tc.tile_pool(name="psA", bufs=1, space="PSUM")
        pA_pool = psum_stack.__enter__()
        qkp_ctx = tc.tile_pool(name="psQK", bufs=1, space="PSUM")
        qkp_pool = qkp_ctx.__enter__()
        av_ctx = tc.tile_pool(name="psAV", bufs=1, space="PSUM")
        av_pool = av_ctx.__enter__()

        def load_xt_quarter(n0, nn):
            """Stream one S-column quarter of x^T (all 8 contraction chunks)."""
            tiles = []
            for c in range(KC):
                t = xt_pool.tile([128, nn], F32R, tag="xt", bufs=KC, name=f"xt{c}")
                nc.sync.dma_start(
                    out=t, in_=io["xT"][c * 128 : (c + 1) * 128, n0 : n0 + nn]
                )
                tiles.append(t)
            return tiles

        # ---- v projection ([seq, chan] layout + ones columns) ----------
        wv_sb = []
        for c in range(KC):
            w = cpool.tile([128, HPC * D], F32R, tag="wv", bufs=KC, name=f"wv{c}")
            nc.sync.dma_start(out=w, in_=io["wvT"][c * 128 : (c + 1) * 128, :])
            wv_sb.append(w)
        out_pair = [
            pair_pool.tile([128, S], F32R, tag="pair", bufs=4, name=f"pair{i}")
            for i in range(4)
        ]
        projw_sb = [
            pw_pool.tile([128, C], F32R, tag="pjw", bufs=4, name=f"pjw{kc}")
            for kc in range(4)
        ]

        # ---- per pair: qk projection + rope, then attention ------------
        # qk tile t: even t -> q of pair t//2, odd t -> k of pair t//2
        # rows 0:64 = first head of the pair, 64:128 = second head.
        for p in range(4):
            qkf_pair = {}
            for t in (2 * p, 2 * p + 1):
                qkf_pair[t] = qkf_pool.tile(
                    [128, S], F32R, tag="qkf", bufs=4, name=f"qkf{t}"
                )
            wqk_s = {}
            for t in (2 * p, 2 * p + 1):
                ws = []
                for c in range(KC):
                    w = wqk_pool.tile(
                        [128, 128], F32R, tag="wqks", bufs=2 * KC,
                        name=f"wqks{c}_{t}",
                    )
                    nc.sync.dma_start(
                        out=w,
                        in_=io["wqkT"][
                            c * 128 : (c + 1) * 128, t * 128 : (t + 1) * 128
                        ],
                    )
                    ws.append(w)
                wqk_s[t] = ws
            for n0, nn in full_chunks:
                xt_q = load_xt_quarter(n0, nn)
                if p == 0 and n0 == 0:
                    nc.sync.dma_start(out=cos2_sb, in_=io["cos2"])
                    nc.sync.dma_start(out=sin2_sb, in_=io["sin2"])
                if p == 0 and n0 == full_chunks[1][0] if len(full_chunks) > 1 else (p == 0 and n0 == 0):
                    # fill only the per-head ones columns (strided, tiny)
                    ones_col = bass.AP(
                        tensor=io["ones65"].tensor,
                        offset=D,
                        ap=[[VW, 128], [0, KT * HPC], [1, 1]],
                    )
                    nc.sync.dma_start(
                        out=v_store.rearrange("p (g u) -> p g u", u=VW)[
                            :, :, D : D + 1
                        ],
                        in_=ones_col,
                    )
                if p == 0:
                    # v projection rides on pair 0's xt stream
                    for ii in range(nn // 128):
                        gi = n0 // 128 + ii
                        pv = pA_pool.tile(
                            [128, HPC * D], F32, tag="pa", bufs=2, name="pv"
                        )
                        for c in range(KC):
                            nc.tensor.matmul(
                                pv,
                                lhsT=xt_q[c][:, ii * 128 : (ii + 1) * 128],
                                rhs=wv_sb[c],
                                start=(c == 0),
                                stop=(c == KC - 1),
                                skip_group_check=True,
                            )
                        dst = v_store[
                            :, gi * HPC * VW : (gi + 1) * HPC * VW
                        ].rearrange("p (h u) -> p h u", u=VW)[:, :, 0:D]
                        nc.vector.tensor_copy(
                            dst, pv.rearrange("p (h u) -> p h u", u=D)
                        )
                for t in (2 * p, 2 * p + 1):
                    pA = pA_pool.tile([128, nn], F32, tag="pa", bufs=2, name="pA")
                    for c in range(KC):
                        nc.tensor.matmul(
                            pA,
                            lhsT=wqk_s[t][c],
                            rhs=xt_q[c],
                            start=(c == 0),
                            stop=(c == KC - 1),
                            skip_group_check=True,
                        )
                    raw = rope_pool.tile([128, nn], F32R, tag="raw", bufs=2, name="raw")
                    nc.vector.tensor_copy(raw, pA)
                    # rotate-half via partition-shifted SBUF copies (DMA)
                    rot = rope_pool.tile([128, nn], F32, tag="rot", bufs=2, name="rot")
                    for blk in range(2):
                        b0 = blk * 64
                        nc.sync.dma_start(
                            out=rot[b0 : b0 + 32, :],
                            in_=raw[b0 + 32 : b0 + 64, :].bitcast(F32),
                        )
                        nc.sync.dma_start(
                            out=rot[b0 + 32 : b0 + 64, :],
                            in_=raw[b0 : b0 + 32, :].bitcast(F32),
                        )
                    # qkf = (pA + bias) * cos2 + (rot + rot_bias) * sin2
                    nc.vector.scalar_tensor_tensor(
                        qkf_pair[t][:, n0 : n0 + nn], pA, qkb_sb[:, t : t + 1],
                        cos2_sb[:, n0 : n0 + nn], op0=OP.add, op1=OP.mult,
                    )
                    t2 = rope_pool.tile([128, nn], F32, tag="t2", bufs=2, name="t2")
                    nc.vector.scalar_tensor_tensor(
                        t2, rot, qkbr_sb[:, t : t + 1],
                        sin2_sb[:, n0 : n0 + nn], op0=OP.add, op1=OP.mult,
                    )
                    nc.vector.tensor_add(
                        qkf_pair[t][:, n0 : n0 + nn],
                        qkf_pair[t][:, n0 : n0 + nn],
                        t2.bitcast(F32R),
                    )

            if p == 2:
                for kc in range(4):
                    nc.sync.dma_start(
                        out=projw_sb[kc],
                        in_=io["projT"][kc * 128 : (kc + 1) * 128, :],
                    )

            # attention for this pair, in q-half passes (2-bank AV psum)
            qT = qkf_pair[2 * p]
            kT = qkf_pair[2 * p + 1]
            for lh in (2 * p, 2 * p + 1):
                r0 = (lh % 2) * 64
                for qp in range(2):
                    q0 = qp * S2
                    avp = av_pool.tile([D + 1, S2], F32, tag="av", bufs=1, name="av")
                    for i in range(KT):
                        qkp = qkp_pool.tile(
                            [128, S2], F32, tag="qkp", bufs=2, name="qkp"
                        )
                        for n0, nn in half_chunks:
                            nc.tensor.matmul(
                                qkp[:, n0 : n0 + nn],
                                lhsT=kT[r0 : r0 + 64, i * 128 : (i + 1) * 128],
                                rhs=qT[r0 : r0 + 64, q0 + n0 : q0 + n0 + nn],
                                start=True,
                                stop=True,
                            )
                        at = attn_pool.tile(
                            [128, S2], F32R, tag="attn", bufs=2, name="at"
                        )
                        nc.scalar.activation(at, qkp, AF.Exp, scale=0.125)
                        vsl = v_store[
                            :, i * HPC * VW + lh * VW : i * HPC * VW + (lh + 1) * VW
                        ]
                        for n0, nn in half_chunks:
                            nc.tensor.matmul(
                                avp[:, n0 : n0 + nn],
                                lhsT=vsl,
                                rhs=at[:, n0 : n0 + nn],
                                start=(i == 0),
                                stop=(i == KT - 1),
                                skip_group_check=True,
                            )
                    # normalize: out = avp[0:64] * (1 / avp[64])
                    outh = oh_pool.tile([64, S2], F32R, tag="outh", bufs=1, name="outh")
                    nc.vector.tensor_copy(outh, avp[0:D, :])
                    stmp = div_pool.tile([D + 1, S2], F32, tag="stmp", bufs=1, name="stmp")
                    nc.vector.tensor_copy(stmp[D : D + 1, :], avp[D : D + 1, :])
                    stmp0 = div_pool.tile([1, S2], F32, tag="stmp0", bufs=1, name="stmp0")
                    nc.sync.dma_start(out=stmp0, in_=stmp[D : D + 1, :])
                    nc.vector.reciprocal_approx_fast(stmp0, stmp0)
                    rbc = div_pool.tile([64, S2], F32, tag="rbc", bufs=1, name="rbc")
                    nc.gpsimd.partition_broadcast(rbc, stmp0)
                    nc.vector.tensor_mul(outh, outh, rbc.bitcast(F32R))
                    nc.sync.dma_start(
                        out=out_pair[p][r0 : r0 + 64, q0 : q0 + S2], in_=outh
                    )

        av_ctx.__exit__(None, None, None)
        qkp_ctx.__exit__(None, None, None)
        psum_stack.__exit__(None, None, None)
        transient.close()

        # ---- output projection -----------------------------------------
        with (
            tc.tile_pool(name="yps", bufs=1, space="PSUM") as yp_pool,
            tc.tile_pool(name="ysbp", bufs=1) as ysb_pool,
        ):
            for m in range(8):
                yp = yp_pool.tile([128, S], F32, tag="yp", bufs=2, name="yp")
                for kc in range(4):
                    for n0, nn in full_chunks:
                        nc.tensor.matmul(
                            yp[:, n0 : n0 + nn],
                            lhsT=projw_sb[kc][:, m * 128 : (m + 1) * 128],
                            rhs=out_pair[kc][:, n0 : n0 + nn],
                            start=(kc == 0),
                            stop=(kc == 3),
                            skip_group_check=True,
                        )
                for hf in range(2):
                    sl = slice(hf * S2, (hf + 1) * S2)
                    ysb = ysb_pool.tile([128, S2], F32, tag="ysb", bufs=2, name="ysb")
                    nc.vector.tensor_copy(ysb, yp[:, sl])
                    nc.sync.dma_start(
                        out=io["yT"][m * 128 : (m + 1) * 128, sl], in_=ysb
                    )


def build(S=2048):
    nc = bacc.Bacc("TRN2", target_bir_lowering=False, debug=False)
    io = {
        "xT": nc.dram_tensor("xT", [C, S], F32R, kind="ExternalInput").ap(),
        "wqkT": nc.dram_tensor("wqkT", [C, 2 * HPC * D], F32R, kind="ExternalInput").ap(),
        "wvT": nc.dram_tensor("wvT", [C, HPC * D], F32R, kind="ExternalInput").ap(),
        "projT": nc.dram_tensor("projT", [HPC * D, C], F32R, kind="ExternalInput").ap(),
        "cos2": nc.dram_tensor("cos2", [128, S], F32, kind="ExternalInput").ap(),
        "sin2": nc.dram_tensor("sin2", [128, S], F32, kind="ExternalInput").ap(),
        "r2t": nc.dram_tensor("r2t", [128, 128], F32R, kind="ExternalInput").ap(),
        "ones65": nc.dram_tensor("ones65", [128, VW], F32R, kind="ExternalInput").ap(),
        "qkb": nc.dram_tensor("qkb", [128, 8], F32, kind="ExternalInput").ap(),
        "qkbr": nc.dram_tensor("qkbr", [128, 8], F32, kind="ExternalInput").ap(),
        "yT": nc.dram_tensor("yT", [C, S], F32, kind="ExternalOutput").ap(),
    }
    with tile.TileContext(nc) as tc:
        _emit(tc, io, S)
    nc.compile()
    return nc


def _sigma():
    """rotate-half permutation on 128 rows (two stacked 64-channel heads)."""
    m = np.arange(128)
    return (m // 64) * 64 + (m % 64 + 32) % 64


def make_core_inputs(core, x, qkv_w, q_bias, proj_w, rope_sin, rope_cos):
    """Build the host-side sharded/transposed input dict for one core."""
    S = x.shape[1]
    b, hg = core // 2, core % 2
    f32 = np.float32

    xT = np.ascontiguousarray(x[b].T, dtype=f32)

    blocks = []
    for p in range(4):
        h0 = hg * HPC + 2 * p
        blocks.append(qkv_w[h0 * D : (h0 + 2) * D, :])  # q rows, heads h0, h0+1
        blocks.append(qkv_w[C + h0 * D : C + (h0 + 2) * D, :])  # k rows
    wqkT = np.ascontiguousarray(np.concatenate(blocks, axis=0).T, dtype=f32)

    wvT = np.ascontiguousarray(
        qkv_w[2 * C + hg * HPC * D : 2 * C + (hg + 1) * HPC * D, :].T, dtype=f32
    )
    projT = np.ascontiguousarray(
        proj_w[:, hg * HPC * D : (hg + 1) * HPC * D].T, dtype=f32
    )

    c1 = np.ones((D, S), dtype=f32)
    c1[:, 1:] = rope_cos.T
    cos2 = np.ascontiguousarray(np.vstack([c1, c1]))
    s1 = np.zeros((D, S), dtype=f32)
    s1[:, 1:] = rope_sin.T
    s1[:32, :] *= -1.0
    sin2 = np.ascontiguousarray(np.vstack([s1, s1]))

    sig = _sigma()
    r2t = np.zeros((128, 128), dtype=f32)
    r2t[sig, np.arange(128)] = 1.0

    qkb = np.zeros((128, 8), dtype=f32)
    for p in range(4):
        h0 = hg * HPC + 2 * p
        qkb[:, 2 * p] = q_bias[h0 * D : (h0 + 2) * D]
    qkbr = qkb[sig, :].copy()

    return {
        "xT": xT, "wqkT": wqkT, "wvT": wvT, "projT": projT,
        "cos2": cos2, "sin2": sin2, "r2t": r2t, "qkb": qkb, "qkbr": qkbr,
        "ones65": np.ones((128, VW), dtype=f32),
    }


_PROGRAM = {}


def _get_program(S):
    if S not in _PROGRAM:
        _PROGRAM[S] = build(S)
    return _PROGRAM[S]


def combine_outputs(yT_list, x, v_bias, proj_w, proj_b):
    """Sum per-core partials and add the host-folded bias corrections."""
    S = x.shape[1]
    corr = (
        v_bias.astype(np.float64) @ proj_w.T.astype(np.float64)
        + proj_b.astype(np.float64)
    ).astype(np.float32)
    y = np.empty((B, S, C), dtype=np.float32)
    for b in range(B):
        y[b] = yT_list[2 * b].T + yT_list[2 * b + 1].T + corr
    return y


def kernel(x, qkv_w, q_bias, v_bias, proj_w, proj_b, rope_sin, rope_cos):
    x = np.asarray(x, dtype=np.float32)
    qkv_w = np.asarray(qkv_w, dtype=np.float32)
    q_bias = np.asarray(q_bias, dtype=np.float32)
    v_bias = np.asarray(v_bias, dtype=np.float32)
    proj_w = np.asarray(proj_w, dtype=np.float32)
    proj_b = np.asarray(proj_b, dtype=np.float32)
    rope_sin = np.asarray(rope_sin, dtype=np.float32)
    rope_cos = np.asarray(rope_cos, dtype=np.float32)

    S = x.shape[1]
    in_maps = [
        make_core_inputs(c, x, qkv_w, q_bias, proj_w, rope_sin, rope_cos)
        for c in range(NCORES)
    ]
    nc = _get_program(S)
    res = run_bass_kernel_spmd(nc, in_maps, core_ids=list(range(NCORES)))
    yT_list = [r["yT"] for r in res.results]
    return combine_outputs(yT_list, x, v_bias, proj_w=proj_w, proj_b=proj_b)



# revision 6
# speedup vs baseline: 1.1530x; 1.1530x over previous
"""Trainium2 Bass kernel for EvaAttention (B=4, S=2048, C=1024, H=16, D=64).

Sharding: 8 cores = 4 batches x 2 head-groups (8 heads each). Each core runs
the identical SPMD program on host-sliced inputs.

v2 design (bf16 matmul path):
  - all matmul operands bf16 (PSUM accumulation stays fp32) -> fast weight
    load (FWL) hides LDWEIGHTS, and DVE ops run in 2x packed modes,
  - x^T loaded to SBUF once (bf16, 32KB/partition) instead of re-streamed,
  - qk projection accumulates in PSUM [128,512] chunks; bias applied by a
    DVE tensor_scalar during PSUM->SBUF eviction; RoPE rotate-half via
    SBUF->SBUF partition-block DMAs + two bf16 tensor_tensor multiplies,
  - v projection in [seq, chan] layout with a packed ones-column so the
    attention AV matmul also produces softmax denominators,
  - per head: exp(QK^T) on ACT (scale folded in) -> AV accumulate ->
    normalize via reciprocal + gpsimd partition broadcast,
  - output projection producing the per-core partial y^T (fp32).
Host sums the two head-group partials per batch and adds the bias
corrections (proj bias + v_bias folded through the projection).
"""

import os
import sys

import numpy as np
import ml_dtypes

for _p in ("/opt/trn_rl_repo", "/root/.axon_site/_ro/trn_rl_repo"):
    if os.path.isdir(_p) and _p not in sys.path:
        sys.path.append(_p)

import concourse.bass as bass  # noqa: E402,F401
import concourse.mybir as mybir  # noqa: E402
import concourse.tile as tile  # noqa: E402
from concourse import bacc  # noqa: E402
from concourse.bass_utils import run_bass_kernel_spmd  # noqa: E402

F32 = mybir.dt.float32
BF16 = mybir.dt.bfloat16
F16 = mybir.dt.float16
AF = mybir.ActivationFunctionType
OP = mybir.AluOpType
BF16NP = ml_dtypes.bfloat16

B = 4
C = 1024
D = 64
H = 16
HPC = 8  # heads per core
NCORES = 8
KC = C // 128  # contraction chunks for the projections
VW = D + 1  # v-store block width per head (64 v cols + ones col)
NCH = 512  # matmul free-dim chunk (one PSUM bank of fp32)


def _emit(tc, io, S):
    nc = tc.nc
    KT = S // 128  # k-position tiles
    S2 = S // 2  # attention q-pass width
    NQ = S // NCH  # 512-wide chunks across S

    with (
        tc.tile_pool(name="cst", bufs=1) as cpool,
        tc.tile_pool(name="xtp", bufs=1) as xt_pool,
        tc.tile_pool(name="wp", bufs=1) as w_pool,
        tc.tile_pool(name="vstp", bufs=1) as v_pool,
        tc.tile_pool(name="qkfp", bufs=1) as qkf_pool,
        tc.tile_pool(name="ropep", bufs=1) as rope_pool,
        tc.tile_pool(name="attnp", bufs=1) as attn_pool,
        tc.tile_pool(name="divp", bufs=1) as div_pool,
        tc.tile_pool(name="outp", bufs=1) as out_pool,
        tc.tile_pool(name="ysbp", bufs=1) as ysb_pool,
        tc.tile_pool(name="psA", bufs=1, space="PSUM") as pA_pool,
        tc.tile_pool(name="psQK", bufs=1, space="PSUM") as qkp_pool,
        tc.tile_pool(name="psAV", bufs=1, space="PSUM") as av_pool,
    ):
        # ---- constants + full-kernel-resident tensors -------------------
        qkb_sb = cpool.tile([128, 8], F32, tag="qkb", name="qkb")
        nc.sync.dma_start(out=qkb_sb, in_=io["qkb"])
        cos2_sb = cpool.tile([128, S], F16, tag="cos2", name="cos2")
        nc.sync.dma_start(out=cos2_sb, in_=io["cos2"])
        sin2_sb = cpool.tile([128, S], F16, tag="sin2", name="sin2")
        nc.sync.dma_start(out=sin2_sb, in_=io["sin2"])

        xt_sb = []
        for c in range(KC):
            t = xt_pool.tile([128, S], F16, tag="xt", bufs=KC, name=f"xt{c}")
            nc.sync.dma_start(out=t, in_=io["xT"][c * 128 : (c + 1) * 128, :])
            xt_sb.append(t)

        wv_sb = []
        for c in range(KC):
            w = w_pool.tile([128, HPC * D], F16, tag="wv", bufs=KC, name=f"wv{c}")
            nc.sync.dma_start(out=w, in_=io["wvT"][c * 128 : (c + 1) * 128, :])
            wv_sb.append(w)
        wqk_sb = []
        for c in range(KC):
            w = w_pool.tile([128, C], F16, tag="wqk", bufs=KC, name=f"wqk{c}")
            nc.sync.dma_start(out=w, in_=io["wqkT"][c * 128 : (c + 1) * 128, :])
            wqk_sb.append(w)
        projw_sb = []
        for kc in range(4):
            w = w_pool.tile([128, C], F16, tag="pjw", bufs=4, name=f"pjw{kc}")
            nc.sync.dma_start(out=w, in_=io["projT"][kc * 128 : (kc + 1) * 128, :])
            projw_sb.append(w)

        v_store = v_pool.tile([128, KT * HPC * VW], BF16, tag="vst", name="vst")
        # fill everything with 1.0; the v-projection overwrites the 64-wide
        # value blocks, leaving the per-head ones columns for the softmax
        # denominators
        nc.vector.memset(v_store, 1.0)

        # ---- v projection ([seq, chan] layout) --------------------------
        for gi in range(KT):
            pv = pA_pool.tile([128, HPC * D], F32, tag="pa", bufs=2, name="pv")
            for c in range(KC):
                nc.tensor.matmul(
                    pv,
                    lhsT=xt_sb[c][:, gi * 128 : (gi + 1) * 128],
                    rhs=wv_sb[c],
                    start=(c == 0),
                    stop=(c == KC - 1),
                    skip_group_check=True,
                )
            dst = v_store[:, gi * HPC * VW : (gi + 1) * HPC * VW].rearrange(
                "p (h u) -> p h u", u=VW
            )[:, :, 0:D]
            nc.vector.tensor_copy(dst, pv.rearrange("p (h u) -> p h u", u=D))

        out_pair = [
            out_pool.tile([128, S], F16, tag="pair", bufs=4, name=f"pair{i}")
            for i in range(4)
        ]

        # ---- per pair: qk projection + rope, then attention -------------
        # qk tile t: even t -> q of pair t//2, odd t -> k of pair t//2
        # rows 0:64 = first head of the pair, 64:128 = second head.
        for p in range(4):
            qkf_pair = {}
            for t in (2 * p, 2 * p + 1):
                qkf = qkf_pool.tile([128, S], F16, tag="qkf", bufs=4, name=f"qkf{t}")
                qkf_pair[t] = qkf
                raw = rope_pool.tile([128, S], F16, tag="raw", bufs=2, name="raw")
                for nj in range(NQ):
                    n0 = nj * NCH
                    pA = pA_pool.tile([128, NCH], F32, tag="pa", bufs=2, name="pA")
                    for c in range(KC):
                        nc.tensor.matmul(
                            pA,
                            lhsT=wqk_sb[c][:, t * 128 : (t + 1) * 128],
                            rhs=xt_sb[c][:, n0 : n0 + NCH],
                            start=(c == 0),
                            stop=(c == KC - 1),
                            skip_group_check=True,
                        )
                    # biased eviction: raw = pA + q_bias (bf16)
                    nc.vector.tensor_scalar_add(
                        raw[:, n0 : n0 + NCH], pA, qkb_sb[:, t : t + 1]
                    )
                # rotate-half via partition-block SBUF->SBUF DMAs
                rot = rope_pool.tile([128, S], F16, tag="rot", bufs=2, name="rot")
                for blk in range(2):
                    b0 = blk * 64
                    nc.sync.dma_start(
                        out=rot[b0 : b0 + 32, :], in_=raw[b0 + 32 : b0 + 64, :]
                    )
                    nc.sync.dma_start(
                        out=rot[b0 + 32 : b0 + 64, :], in_=raw[b0 : b0 + 32, :]
                    )
                # qkf = raw*cos2 + rot*sin2 (sign of rotate folded into sin2)
                for nj in range(NQ):
                    sl = slice(nj * NCH, (nj + 1) * NCH)
                    t2 = rope_pool.tile([128, NCH], F16, tag="t2", bufs=3, name="t2")
                    nc.vector.tensor_mul(qkf[:, sl], raw[:, sl], cos2_sb[:, sl])
                    nc.vector.tensor_mul(t2, rot[:, sl], sin2_sb[:, sl])
                    nc.vector.tensor_add(qkf[:, sl], qkf[:, sl], t2)

            # attention for this pair, in q-half passes
            qT = qkf_pair[2 * p]
            kT = qkf_pair[2 * p + 1]
            for lh in range(2):
                r0 = lh * 64
                head = 2 * p + lh  # head slot within the core (0..7)
                for qp in range(2):
                    q0 = qp * S2
                    avp = av_pool.tile([D + 1, S2], F32, tag="av", bufs=1, name="av")
                    for i in range(KT):
                        qkp = qkp_pool.tile(
                            [128, S2], F32, tag="qkp", bufs=2, name="qkp"
                        )
                        for nj in range(S2 // NCH):
                            n0 = nj * NCH
                            nc.tensor.matmul(
                                qkp[:, n0 : n0 + NCH],
                                lhsT=kT[r0 : r0 + 64, i * 128 : (i + 1) * 128],
                                rhs=qT[r0 : r0 + 64, q0 + n0 : q0 + n0 + NCH],
                                start=True,
                                stop=True,
                            )
                        at = attn_pool.tile([128, S2], BF16, tag="attn", bufs=3, name="at")
                        nc.scalar.activation(at, qkp, AF.Exp, scale=0.125)
                        vsl = v_store[
                            :, i * HPC * VW + head * VW : i * HPC * VW + (head + 1) * VW
                        ]
                        for nj in range(S2 // NCH):
                            n0 = nj * NCH
                            nc.tensor.matmul(
                                avp[:, n0 : n0 + NCH],
                                lhsT=vsl,
                                rhs=at[:, n0 : n0 + NCH],
                                start=(i == 0),
                                stop=(i == KT - 1),
                                skip_group_check=True,
                            )
                    # normalize: out = avp[0:64] * (1 / avp[64]) broadcast
                    stmp = div_pool.tile([D + 1, S2], F32, tag="stmp", bufs=2, name="stmp")
                    nc.vector.tensor_copy(stmp[D : D + 1, :], avp[D : D + 1, :])
                    stmp0 = div_pool.tile([1, S2], F32, tag="stmp0", bufs=2, name="stmp0")
                    nc.sync.dma_start(out=stmp0, in_=stmp[D : D + 1, :])
                    nc.vector.reciprocal_approx_fast(stmp0, stmp0)
                    rbc = div_pool.tile([64, S2], F32, tag="rbc", bufs=2, name="rbc")
                    nc.gpsimd.partition_broadcast(rbc, stmp0)
                    outh = div_pool.tile([64, S2], F16, tag="outh", bufs=2, name="outh")
                    nc.vector.tensor_mul(outh, avp[0:D, :], rbc)
                    nc.sync.dma_start(
                        out=out_pair[p][r0 : r0 + 64, q0 : q0 + S2], in_=outh
                    )

        # ---- output projection ------------------------------------------
        for m in range(8):
            for nj in range(NQ):
                n0 = nj * NCH
                yp = pA_pool.tile([128, NCH], F32, tag="pa", bufs=2, name="yp")
                for kc in range(4):
                    nc.tensor.matmul(
                        yp,
                        lhsT=projw_sb[kc][:, m * 128 : (m + 1) * 128],
                        rhs=out_pair[kc][:, n0 : n0 + NCH],
                        start=(kc == 0),
                        stop=(kc == 3),
                        skip_group_check=True,
                    )
                ysb = ysb_pool.tile([128, NCH], F32, tag="ysb", bufs=3, name="ysb")
                nc.vector.tensor_copy(ysb, yp)
                nc.sync.dma_start(
                    out=io["yT"][m * 128 : (m + 1) * 128, n0 : n0 + NCH], in_=ysb
                )


def build(S=2048):
    nc = bacc.Bacc("TRN2", target_bir_lowering=False, debug=False)
    io = {
        "xT": nc.dram_tensor("xT", [C, S], F16, kind="ExternalInput").ap(),
        "wqkT": nc.dram_tensor(
            "wqkT", [C, 2 * HPC * D], F16, kind="ExternalInput"
        ).ap(),
        "wvT": nc.dram_tensor("wvT", [C, HPC * D], F16, kind="ExternalInput").ap(),
        "projT": nc.dram_tensor("projT", [HPC * D, C], F16, kind="ExternalInput").ap(),
        "cos2": nc.dram_tensor("cos2", [128, S], F16, kind="ExternalInput").ap(),
        "sin2": nc.dram_tensor("sin2", [128, S], F16, kind="ExternalInput").ap(),
        "ones65": nc.dram_tensor("ones65", [128, VW], BF16, kind="ExternalInput").ap(),
        "qkb": nc.dram_tensor("qkb", [128, 8], F32, kind="ExternalInput").ap(),
        "yT": nc.dram_tensor("yT", [C, S], F32, kind="ExternalOutput").ap(),
    }
    with tile.TileContext(nc) as tc:
        _emit(tc, io, S)
    nc.compile()
    return nc


def make_core_inputs(core, x, qkv_w, q_bias, proj_w, rope_sin, rope_cos):
    """Build the host-side sharded/transposed input dict for one core."""
    S = x.shape[1]
    b, hg = core // 2, core % 2
    f32 = np.float32

    xT = np.ascontiguousarray(x[b].T).astype(np.float16)

    blocks = []
    for p in range(4):
        h0 = hg * HPC + 2 * p
        blocks.append(qkv_w[h0 * D : (h0 + 2) * D, :])  # q rows, heads h0, h0+1
        blocks.append(qkv_w[C + h0 * D : C + (h0 + 2) * D, :])  # k rows
    wqkT = np.ascontiguousarray(np.concatenate(blocks, axis=0).T).astype(np.float16)

    wvT = np.ascontiguousarray(
        qkv_w[2 * C + hg * HPC * D : 2 * C + (hg + 1) * HPC * D, :].T
    ).astype(np.float16)
    projT = np.ascontiguousarray(
        proj_w[:, hg * HPC * D : (hg + 1) * HPC * D].T
    ).astype(np.float16)

    c1 = np.ones((D, S), dtype=f32)
    c1[:, 1:] = rope_cos.T
    cos2 = np.ascontiguousarray(np.vstack([c1, c1])).astype(np.float16)
    s1 = np.zeros((D, S), dtype=f32)
    s1[:, 1:] = rope_sin.T
    s1[:32, :] *= -1.0
    sin2 = np.ascontiguousarray(np.vstack([s1, s1])).astype(np.float16)

    qkb = np.zeros((128, 8), dtype=f32)
    for p in range(4):
        h0 = hg * HPC + 2 * p
        qkb[:, 2 * p] = q_bias[h0 * D : (h0 + 2) * D]

    return {
        "xT": xT,
        "wqkT": wqkT,
        "wvT": wvT,
        "projT": projT,
        "cos2": cos2,
        "sin2": sin2,
        "qkb": qkb,
        "ones65": np.ones((128, VW), dtype=BF16NP),
    }


_PROGRAM = {}


def _get_program(S):
    if S not in _PROGRAM:
        _PROGRAM[S] = build(S)
    return _PROGRAM[S]


def combine_outputs(yT_list, x, v_bias, proj_w, proj_b):
    """Sum per-core partials and add the host-folded bias corrections."""
    S = x.shape[1]
    corr = (
        v_bias.astype(np.float64) @ proj_w.T.astype(np.float64)
        + proj_b.astype(np.float64)
    ).astype(np.float32)
    y = np.empty((B, S, C), dtype=np.float32)
    for b in range(B):
        y[b] = yT_list[2 * b].T + yT_list[2 * b + 1].T + corr
    return y


def kernel(x, qkv_w, q_bias, v_bias, proj_w, proj_b, rope_sin, rope_cos):
    x = np.asarray(x, dtype=np.float32)
    qkv_w = np.asarray(qkv_w, dtype=np.float32)
    q_bias = np.asarray(q_bias, dtype=np.float32)
    v_bias = np.asarray(v_bias, dtype=np.float32)
    proj_w = np.asarray(proj_w, dtype=np.float32)
    proj_b = np.asarray(proj_b, dtype=np.float32)
    rope_sin = np.asarray(rope_sin, dtype=np.float32)
    rope_cos = np.asarray(rope_cos, dtype=np.float32)

    S = x.shape[1]
    in_maps = [
        make_core_inputs(c, x, qkv_w, q_bias, proj_w, rope_sin, rope_cos)
        for c in range(NCORES)
    ]
    nc = _get_program(S)
    res = run_bass_kernel_spmd(nc, in_maps, core_ids=list(range(NCORES)))
    yT_list = [r["yT"] for r in res.results]
    return combine_outputs(yT_list, x, v_bias, proj_w=proj_w, proj_b=proj_b)


# revision 13
# speedup vs baseline: 1.1766x; 1.0205x over previous
"""Trainium2 Bass kernel for EvaAttention (B=4, S=2048, C=1024, H=16, D=64).

Sharding: 8 cores = 4 batches x 2 head-groups (8 heads each). Each core runs
the identical SPMD program on host-sliced inputs.

v3 design (fp16 qk path, bf16 attention probs, interleaved emission):
  - matmul operands 16-bit (fp16 for the q/k/projection path for precision,
    bf16 for exp outputs for range); PSUM accumulation stays fp32,
  - x^T resident in SBUF (fp16); all weights resident,
  - attention is ACT(exp)-bound at ~1.1us per k-tile; projection matmuls are
    emitted interleaved into the attention stream (1 filler MM per k-tile)
    so the tensor engine uses the exp slack instead of serializing phases,
  - v projection is interleaved into the first attention unit (8 MMs per
    k-tile, just ahead of the AV matmul that consumes each chunk),
  - RoPE: bias applied during PSUM eviction (tensor_scalar), rotate-half by
    partition-block SBUF->SBUF DMAs, combine with two fp16 tensor_tensor ops,
  - softmax denominators ride the AV matmul via a ones-column in v_store;
    normalize = psum-direct reciprocal + gpsimd partition broadcast,
  - output projection as a short tail phase (contraction over all 4 pairs).
Host sums the two head-group partials per batch and adds the bias
corrections (proj bias + v_bias folded through the projection).
"""

import os
import sys
from collections import deque

import numpy as np
import ml_dtypes

for _p in ("/opt/trn_rl_repo", "/root/.axon_site/_ro/trn_rl_repo"):
    if os.path.isdir(_p) and _p not in sys.path:
        sys.path.append(_p)

import concourse.bass as bass  # noqa: E402,F401
import concourse.mybir as mybir  # noqa: E402
import concourse.tile as tile  # noqa: E402
from concourse import bacc  # noqa: E402
from concourse.bass_utils import run_bass_kernel_spmd  # noqa: E402

F32 = mybir.dt.float32
BF16 = mybir.dt.bfloat16
F16 = mybir.dt.float16
AF = mybir.ActivationFunctionType
OP = mybir.AluOpType
BF16NP = ml_dtypes.bfloat16

B = 4
C = 1024
D = 64
H = 16
HPC = 8  # heads per core
NCORES = 8
KC = C // 128  # contraction chunks for the projections
VW = D + 1  # v-store block width per head (64 v cols + ones col)
NCH = 512  # matmul free-dim chunk (one PSUM bank of fp32)


def _emit(tc, io, S):
    nc = tc.nc
    KT = S // 128  # k-position tiles
    S2 = S // 2  # attention q-pass width
    NQ = S // NCH

    with (
        tc.tile_pool(name="cst", bufs=1) as cpool,
        tc.tile_pool(name="xtp", bufs=1) as xt_pool,
        tc.tile_pool(name="wp", bufs=1) as w_pool,
        tc.tile_pool(name="vstp", bufs=1) as v_pool,
        tc.tile_pool(name="qkfp", bufs=1) as qkf_pool,
        tc.tile_pool(name="ropep", bufs=1) as rope_pool,
        tc.tile_pool(name="attnp", bufs=1) as attn_pool,
        tc.tile_pool(name="divp", bufs=1) as div_pool,
        tc.tile_pool(name="outp", bufs=1) as out_pool,
        tc.tile_pool(name="ysbp", bufs=1) as ysb_pool,
        tc.tile_pool(name="psA", bufs=1, space="PSUM") as pA_pool,
        tc.tile_pool(name="psQK", bufs=1, space="PSUM") as qkp_pool,
        tc.tile_pool(name="psAV", bufs=1, space="PSUM") as av_pool,
    ):
        # ---- constants + resident tensors -------------------------------
        qkb_sb = cpool.tile([128, 8], F32, tag="qkb", name="qkb")
        nc.sync.dma_start(out=qkb_sb, in_=io["qkb"])
        cos2_sb = cpool.tile([128, S], F16, tag="cos2", name="cos2")
        nc.sync.dma_start(out=cos2_sb, in_=io["cos2"])
        sin2_sb = cpool.tile([128, S], F16, tag="sin2", name="sin2")
        nc.sync.dma_start(out=sin2_sb, in_=io["sin2"])

        xt_sb = []
        for c in range(KC):
            t = xt_pool.tile([128, S], F16, tag="xt", bufs=KC, name=f"xt{c}")
            nc.sync.dma_start(out=t, in_=io["xT"][c * 128 : (c + 1) * 128, :])
            xt_sb.append(t)
        wv_sb = []
        for c in range(KC):
            w = w_pool.tile([128, HPC * D], F16, tag="wv", bufs=KC, name=f"wv{c}")
            nc.sync.dma_start(out=w, in_=io["wvT"][c * 128 : (c + 1) * 128, :])
            wv_sb.append(w)
        wqk_sb = []
        for c in range(KC):
            w = w_pool.tile([128, C], F16, tag="wqk", bufs=KC, name=f"wqk{c}")
            nc.sync.dma_start(out=w, in_=io["wqkT"][c * 128 : (c + 1) * 128, :])
            wqk_sb.append(w)
        projw_sb = []
        for kc in range(4):
            w = w_pool.tile([128, C], F16, tag="pjw", bufs=4, name=f"pjw{kc}")
            nc.sync.dma_start(out=w, in_=io["projT"][kc * 128 : (kc + 1) * 128, :])
            projw_sb.append(w)

        v_store = v_pool.tile([128, KT * HPC * VW], BF16, tag="vst", name="vst")
        # ones everywhere; v-projection overwrites the 64-wide value blocks,
        # leaving the per-head ones columns (softmax denominators)
        nc.vector.memset(v_store, 1.0)

        qkf_tiles = {}
        out_pair = [
            out_pool.tile([128, S], F16, tag="pair", bufs=4, name=f"pair{i}")
            for i in range(4)
        ]

        # ---- stream-B generators (filler PE work) -----------------------
        def gen_vproj():
            """v projection; one yield per matmul. gi chunk ready after 8."""
            for gi in range(KT):
                pv = pA_pool.tile([128, HPC * D], F32, tag="pa", bufs=2, name="pv")
                for c in range(KC):
                    nc.tensor.matmul(
                        pv,
                        lhsT=xt_sb[c][:, gi * 128 : (gi + 1) * 128],
                        rhs=wv_sb[c],
                        start=(c == 0),
                        stop=(c == KC - 1),
                        skip_group_check=True,
                    )
                    yield
                dst = v_store[:, gi * HPC * VW : (gi + 1) * HPC * VW].rearrange(
                    "p (h u) -> p h u", u=VW
                )[:, :, 0:D]
                nc.vector.tensor_copy(dst, pv.rearrange("p (h u) -> p h u", u=D))

        def gen_qkproj(p):
            """qk projection + rope for pair p; one yield per matmul."""
            for t in (2 * p, 2 * p + 1):
                qkf = qkf_pool.tile([128, S], F16, tag="qkf", bufs=4, name=f"qkf{t}")
                qkf_tiles[t] = qkf
                raw = rope_pool.tile([128, S], F16, tag="raw", bufs=2, name="raw")
                for nj in range(NQ):
                    n0 = nj * NCH
                    pA = pA_pool.tile([128, NCH], F32, tag="pa", bufs=2, name="pA")
                    for c in range(KC):
                        nc.tensor.matmul(
                            pA,
                            lhsT=wqk_sb[c][:, t * 128 : (t + 1) * 128],
                            rhs=xt_sb[c][:, n0 : n0 + NCH],
                            start=(c == 0),
                            stop=(c == KC - 1),
                            skip_group_check=True,
                        )
                        yield
                    nc.vector.tensor_scalar_add(
                        raw[:, n0 : n0 + NCH], pA, qkb_sb[:, t : t + 1]
                    )
                rot = rope_pool.tile([128, S], F16, tag="rot", bufs=2, name="rot")
                for blk in range(2):
                    b0 = blk * 64
                    nc.sync.dma_start(
                        out=rot[b0 : b0 + 32, :], in_=raw[b0 + 32 : b0 + 64, :]
                    )
                    nc.sync.dma_start(
                        out=rot[b0 + 32 : b0 + 64, :], in_=raw[b0 : b0 + 32, :]
                    )
                for nj in range(NQ):
                    sl = slice(nj * NCH, (nj + 1) * NCH)
                    t2 = rope_pool.tile([128, NCH], F16, tag="t2", bufs=3, name="t2")
                    nc.vector.tensor_mul(qkf[:, sl], raw[:, sl], cos2_sb[:, sl])
                    nc.vector.tensor_mul(t2, rot[:, sl], sin2_sb[:, sl])
                    nc.vector.tensor_add(qkf[:, sl], qkf[:, sl], t2)

        def drain(g):
            for _ in g:
                pass

        bq = deque([(1, gen_qkproj(1)), (2, gen_qkproj(2)), (3, gen_qkproj(3))])
        done_pairs = {0}

        def pump(n):
            while n > 0 and bq:
                try:
                    next(bq[0][1])
                    n -= 1
                except StopIteration:
                    done_pairs.add(bq[0][0])
                    bq.popleft()

        # ---- prologue: pair-0 projections (serial) ----------------------
        drain(gen_qkproj(0))
        vgen = gen_vproj()
        if True:  # BISECT: serial mode
            drain(vgen)

        # ---- attention: ACT-paced; stream B interleaved -----------------
        for p in range(4):
            while p not in done_pairs:
                # pair p's qk tiles must be fully emitted before its attention
                pid, g = bq.popleft()
                drain(g)
                done_pairs.add(pid)
            qT = qkf_tiles[2 * p]
            kT = qkf_tiles[2 * p + 1]
            for lh in range(2):
                r0 = lh * 64
                head = 2 * p + lh
                for qp in range(2):
                    q0 = qp * S2
                    first_unit = p == 0 and lh == 0 and qp == 0
                    avp = av_pool.tile([D + 1, S2], F32, tag="av", bufs=1, name="av")
                    for i in range(KT):
                        if first_unit:
                            pass  # BISECT: v-proj drained upfront
                        else:
                            pass  # BISECT: no pumping
                        qkp = qkp_pool.tile(
                            [128, S2], F32, tag="qkp", bufs=2, name="qkp"
                        )
                        for nj in range(S2 // NCH):
                            n0 = nj * NCH
                            nc.tensor.matmul(
                                qkp[:, n0 : n0 + NCH],
                                lhsT=kT[r0 : r0 + 64, i * 128 : (i + 1) * 128],
                                rhs=qT[r0 : r0 + 64, q0 + n0 : q0 + n0 + NCH],
                                start=True,
                                stop=True,
                            )
                        at = attn_pool.tile(
                            [128, S2], BF16, tag="attn", bufs=3, name="at"
                        )
                        nc.scalar.activation(at, qkp, AF.Exp, scale=0.125)
                        vsl = v_store[
                            :,
                            i * HPC * VW + head * VW : i * HPC * VW + (head + 1) * VW,
                        ]
                        for nj in range(S2 // NCH):
                            n0 = nj * NCH
                            nc.tensor.matmul(
                                avp[:, n0 : n0 + NCH],
                                lhsT=vsl,
                                rhs=at[:, n0 : n0 + NCH],
                                start=(i == 0),
                                stop=(i == KT - 1),
                                skip_group_check=True,
                            )
                    # normalize: out = avp[0:64] * (1 / avp[64]) broadcast
                    stmp = div_pool.tile(
                        [D + 1, S2], F32, tag="stmp", bufs=2, name="stmp"
                    )
                    nc.vector.tensor_copy(stmp[D : D + 1, :], avp[D : D + 1, :])
                    stmp0 = div_pool.tile([1, S2], F32, tag="stmp0", bufs=2, name="s0")
                    nc.sync.dma_start(out=stmp0, in_=stmp[D : D + 1, :])
                    nc.vector.reciprocal_approx_fast(stmp0, stmp0)
                    rbc = div_pool.tile([64, S2], F32, tag="rbc", bufs=2, name="rbc")
                    nc.gpsimd.partition_broadcast(rbc, stmp0)
                    outh = div_pool.tile(
                        [64, S2], F16, tag="outh", bufs=2, name="outh"
                    )
                    nc.vector.tensor_mul(outh, avp[0:D, :], rbc)
                    nc.sync.dma_start(
                        out=out_pair[p][r0 : r0 + 64, q0 : q0 + S2], in_=outh
                    )
                    pass  # BISECT: no boundary pump

        while bq:
            drain(bq.popleft()[1])
        drain(vgen)

        # ---- output projection (tail) -----------------------------------
        for m in range(8):
            for nj in range(NQ):
                n0 = nj * NCH
                yp = pA_pool.tile([128, NCH], F32, tag="pa", bufs=2, name="yp")
                for kc in range(4):
                    nc.tensor.matmul(
                        yp,
                        lhsT=projw_sb[kc][:, m * 128 : (m + 1) * 128],
                        rhs=out_pair[kc][:, n0 : n0 + NCH],
                        start=(kc == 0),
                        stop=(kc == 3),
                        skip_group_check=True,
                    )
                ysb = ysb_pool.tile([128, NCH], F32, tag="ysb", bufs=3, name="ysb")
                nc.vector.tensor_copy(ysb, yp)
                nc.sync.dma_start(
                    out=io["yT"][m * 128 : (m + 1) * 128, n0 : n0 + NCH], in_=ysb
                )


def build(S=2048):
    nc = bacc.Bacc("TRN2", target_bir_lowering=False, debug=False)
    io = {
        "xT": nc.dram_tensor("xT", [C, S], F16, kind="ExternalInput").ap(),
        "wqkT": nc.dram_tensor("wqkT", [C, 2 * HPC * D], F16, kind="ExternalInput").ap(),
        "wvT": nc.dram_tensor("wvT", [C, HPC * D], F16, kind="ExternalInput").ap(),
        "projT": nc.dram_tensor("projT", [HPC * D, C], F16, kind="ExternalInput").ap(),
        "cos2": nc.dram_tensor("cos2", [128, S], F16, kind="ExternalInput").ap(),
        "sin2": nc.dram_tensor("sin2", [128, S], F16, kind="ExternalInput").ap(),
        "qkb": nc.dram_tensor("qkb", [128, 8], F32, kind="ExternalInput").ap(),
        "yT": nc.dram_tensor("yT", [C, S], F32, kind="ExternalOutput").ap(),
    }
    with tile.TileContext(nc) as tc:
        _emit(tc, io, S)
    nc.compile()
    return nc


def make_core_inputs(core, x, qkv_w, q_bias, proj_w, rope_sin, rope_cos):
    """Build the host-side sharded/transposed input dict for one core."""
    S = x.shape[1]
    b, hg = core // 2, core % 2
    f32 = np.float32

    xT = np.ascontiguousarray(x[b].T).astype(np.float16)

    blocks = []
    for p in range(4):
        h0 = hg * HPC + 2 * p
        blocks.append(qkv_w[h0 * D : (h0 + 2) * D, :])  # q rows, heads h0, h0+1
        blocks.append(qkv_w[C + h0 * D : C + (h0 + 2) * D, :])  # k rows
    wqkT = np.ascontiguousarray(np.concatenate(blocks, axis=0).T).astype(np.float16)

    wvT = np.ascontiguousarray(
        qkv_w[2 * C + hg * HPC * D : 2 * C + (hg + 1) * HPC * D, :].T
    ).astype(np.float16)
    projT = np.ascontiguousarray(
        proj_w[:, hg * HPC * D : (hg + 1) * HPC * D].T
    ).astype(np.float16)

    c1 = np.ones((D, S), dtype=f32)
    c1[:, 1:] = rope_cos.T
    cos2 = np.ascontiguousarray(np.vstack([c1, c1])).astype(np.float16)
    s1 = np.zeros((D, S), dtype=f32)
    s1[:, 1:] = rope_sin.T
    s1[:32, :] *= -1.0
    sin2 = np.ascontiguousarray(np.vstack([s1, s1])).astype(np.float16)

    qkb = np.zeros((128, 8), dtype=f32)
    for p in range(4):
        h0 = hg * HPC + 2 * p
        qkb[:, 2 * p] = q_bias[h0 * D : (h0 + 2) * D]

    return {
        "xT": xT,
        "wqkT": wqkT,
        "wvT": wvT,
        "projT": projT,
        "cos2": cos2,
        "sin2": sin2,
        "qkb": qkb,
    }


_PROGRAM = {}


def _get_program(S):
    if S not in _PROGRAM:
        _PROGRAM[S] = build(S)
    return _PROGRAM[S]


def combine_outputs(yT_list, x, v_bias, proj_w, proj_b):
    """Sum per-core partials and add the host-folded bias corrections."""
    S = x.shape[1]
    corr = (
        v_bias.astype(np.float64) @ proj_w.T.astype(np.float64)
        + proj_b.astype(np.float64)
    ).astype(np.float32)
    y = np.empty((B, S, C), dtype=np.float32)
    for b in range(B):
        y[b] = yT_list[2 * b].T + yT_list[2 * b + 1].T + corr
    return y


def kernel(x, qkv_w, q_bias, v_bias, proj_w, proj_b, rope_sin, rope_cos):
    x = np.asarray(x, dtype=np.float32)
    qkv_w = np.asarray(qkv_w, dtype=np.float32)
    q_bias = np.asarray(q_bias, dtype=np.float32)
    v_bias = np.asarray(v_bias, dtype=np.float32)
    proj_w = np.asarray(proj_w, dtype=np.float32)
    proj_b = np.asarray(proj_b, dtype=np.float32)
    rope_sin = np.asarray(rope_sin, dtype=np.float32)
    rope_cos = np.asarray(rope_cos, dtype=np.float32)

    S = x.shape[1]
    in_maps = [
        make_core_inputs(c, x, qkv_w, q_bias, proj_w, rope_sin, rope_cos)
        for c in range(NCORES)
    ]
    nc = _get_program(S)
    res = run_bass_kernel_spmd(nc, in_maps, core_ids=list(range(NCORES)))
    yT_list = [r["yT"] for r in res.results]
    return combine_outputs(yT_list, x, v_bias, proj_w=proj_w, proj_b=proj_b)
